# revision 3
# baseline (speedup 1.0000x reference)
"""Trainium2 Bass kernel for nn_DepthAwareEPIBranch.

Reference computation (B=2, C=128, H=W=320, angRes=5):
  xe  = angular rearrange: each contiguous 5x5 block of the image is an
        independent "angular patch" (B*h*w patches, h=w=64).
  eh  = pw(lrelu(dwconv_1x5(xe)), w_h_pw)   # taps masked at 5-block bounds
  ev  = pw(lrelu(dwconv_5x1(xe)), w_v_pw)
  epi = pw(concat(eh, ev), w_fuse)
  dw  = sigmoid(pw(lrelu(pw(epi, w_dm1)), w_dm2))
  out = x + scale * epi * dw

Algebraic folds used here (host-side weight prep):
  - eh/ev only feed the fuse conv:  epi = A_h @ lrelu(dh) + A_v @ lrelu(dv)
    with A_h = w_fuse[:, :C] @ w_h_pw, A_v = w_fuse[:, C:] @ w_v_pw.
  - scale folded into A_* (epi' = scale*epi) and w_dm2' = w_dm2/scale
    (lrelu is positive-homogeneous), so out = x + epi' * sigmoid(...).

Sharding: data-parallel over B*H rows at angular-group granularity.
640 rows = 128 groups of 5; each of 8 cores takes 16 groups (80 rows).

On-chip layout: channels C=128 = SBUF partitions; pixels on the free dim.
Depthwise conv taps are diagonal-weight matmuls accumulating in PSUM
(bf16 operands; fp32 PSUM).
"""

import numpy as np

import concourse.bacc as bacc
import concourse.mybir as mybir
from concourse import tile
from concourse.bass_utils import run_bass_kernel_spmd

F32 = mybir.dt.float32
BF16 = mybir.dt.bfloat16
AF = mybir.ActivationFunctionType
ALU = mybir.AluOpType

P = 128          # channels = partitions
A = 5            # angRes
W = 320          # image width
NB = W // A      # 64 angular blocks per row
RPC = 80         # rows per core (B*H / 8 = 640/8)
NG = RPC // A    # 16 angular row-groups per core
N_CORES = 8

# taps: (weight index k, offset d = k-2); conv out[j] += w[k] * x[j+k-2]
TAPS = [(k, k - 2) for k in range(A)]


def _build_nc():
    nc = bacc.Bacc("TRN2", target_bir_lowering=False, debug=False)

    xs = nc.dram_tensor("xs", [P, RPC, W], F32, kind="ExternalInput")
    # 10 diagonal lhsT matrices: [c_in, k, c_out], k=0..4 horiz, 5..9 vert
    wdiag = nc.dram_tensor("wdiag", [P, 2 * A, P], BF16, kind="ExternalInput")
    aw = nc.dram_tensor("aw", [P, 2, P], BF16, kind="ExternalInput")  # A_h^T, A_v^T
    w1t = nc.dram_tensor("w1t", [P, 32], BF16, kind="ExternalInput")
    w2t = nc.dram_tensor("w2t", [32, P], BF16, kind="ExternalInput")
    ys = nc.dram_tensor("ys", [P, RPC, W], F32, kind="ExternalOutput")

    with tile.TileContext(nc) as tc:
        with (
            tc.tile_pool(name="consts", bufs=1) as cp,
            tc.tile_pool(name="xin", bufs=3) as xp,
            tc.tile_pool(name="xbf", bufs=2) as xbp,
            tc.tile_pool(name="mid", bufs=3) as mp,
            tc.tile_pool(name="outp", bufs=2) as op,
            tc.tile_pool(name="ps2", bufs=2, space="PSUM") as pp2,
            tc.tile_pool(name="ps1", bufs=1, space="PSUM") as pp1,
        ):
            wdiag_t = cp.tile([P, 2 * A, P], BF16)
            nc.sync.dma_start(wdiag_t[:], wdiag[:])
            aw_t = cp.tile([P, 2, P], BF16)
            nc.sync.dma_start(aw_t[:], aw[:])
            w1t_t = cp.tile([P, 32], BF16)
            nc.sync.dma_start(w1t_t[:], w1t[:])
            w2t_t = cp.tile([32, P], BF16)
            nc.sync.dma_start(w2t_t[:], w2t[:])

            for g in range(NG):
                x_t = xp.tile([P, A, W], F32, tag="x")
                nc.sync.dma_start(x_t[:], xs[:, A * g : A * g + A, :])
                xb_t = xbp.tile([P, A, W], BF16, tag="xb")
                nc.vector.tensor_copy(xb_t[:], x_t[:])
                xbv = xb_t[:].rearrange("p r (b q) -> p r b q", q=A)
                out_t = op.tile([P, A, W], F32, tag="out")

                for r in range(A):
                    # ---- horizontal depthwise conv (taps masked per block)
                    dh = pp2.tile([P, NB, A], F32, tag="dh")
                    first = True
                    for k, d in TAPS:
                        if d == 0:
                            o_ap, i_ap = dh[:, :, :], xbv[:, r, :, :]
                        elif d > 0:
                            o_ap = dh[:, :, 0 : A - d]
                            i_ap = xbv[:, r, :, d:A]
                        else:
                            o_ap = dh[:, :, -d:A]
                            i_ap = xbv[:, r, :, 0 : A + d]
                        nc.tensor.matmul(
                            o_ap, wdiag_t[:, k, :], i_ap,
                            start=first, stop=(k == A - 1),
                        )
                        first = False
                    # ---- vertical depthwise conv (whole-row taps, masked by group)
                    dv = pp2.tile([P, NB, A], F32, tag="dv")
                    vtaps = [(k, d) for k, d in TAPS if 0 <= r + d < A]
                    for i, (k, d) in enumerate(vtaps):
                        nc.tensor.matmul(
                            dv[:, :, :], wdiag_t[:, A + k, :], xbv[:, r + d, :, :],
                            start=(i == 0), stop=(i == len(vtaps) - 1),
                        )
                    # ---- leaky relu -> bf16 SBUF
                    lh = mp.tile([P, W], BF16, tag="lh")
                    nc.scalar.activation(
                        lh[:], dh[:].rearrange("p b q -> p (b q)"), AF.Lrelu, alpha=0.1
                    )
                    lv = mp.tile([P, W], BF16, tag="lv")
                    nc.scalar.activation(
                        lv[:], dv[:].rearrange("p b q -> p (b q)"), AF.Lrelu, alpha=0.1
                    )
                    # ---- epi' = A_h @ lh + A_v @ lv
                    epi = pp2.tile([P, W], F32, tag="epi")
                    nc.tensor.matmul(epi[:], aw_t[:, 0, :], lh[:], start=True, stop=False)
                    nc.tensor.matmul(epi[:], aw_t[:, 1, :], lv[:], start=False, stop=True)
                    epi_s = mp.tile([P, W], BF16, tag="epis")
                    nc.scalar.copy(epi_s[:], epi[:])
                    # ---- depth-weight chain
                    m1 = pp1.tile([32, W], F32, tag="m1")
                    nc.tensor.matmul(m1[:], w1t_t[:], epi_s[:], start=True, stop=True)
                    lr1 = mp.tile([32, W], BF16, tag="lr1")
                    nc.scalar.activation(lr1[:], m1[:], AF.Lrelu, alpha=0.1)
                    m2 = pp1.tile([P, W], F32, tag="m2")
                    nc.tensor.matmul(m2[:], w2t_t[:], lr1[:], start=True, stop=True)
                    sg = mp.tile([P, W], BF16, tag="sg")
                    nc.scalar.activation(sg[:], m2[:], AF.Sigmoid)
                    # ---- out = x + epi' * sigmoid
                    prod = mp.tile([P, W], BF16, tag="prod")
                    nc.vector.tensor_tensor(prod[:], epi_s[:], sg[:], ALU.mult)
                    nc.vector.tensor_tensor(
                        out_t[:, r, :], prod[:], x_t[:, r, :], ALU.add
                    )

                nc.sync.dma_start(ys[:, A * g : A * g + A, :], out_t[:])

    nc.compile()
    return nc


_NC_CACHE = None


def _get_nc():
    global _NC_CACHE
    if _NC_CACHE is None:
        _NC_CACHE = _build_nc()
    return _NC_CACHE


def _prep_weights(w_h_dw, w_h_pw, w_v_dw, w_v_pw, w_dm1, w_dm2, w_fuse, scale):
    """Host-side weight folding; returns the per-core (shared) weight arrays."""

    import ml_dtypes

    def bf(x):
        return np.asarray(x, np.float32).astype(ml_dtypes.bfloat16)

    wh = np.asarray(w_h_dw, np.float32).reshape(P, A)      # [c, k]
    wv = np.asarray(w_v_dw, np.float32).reshape(P, A)
    whp = np.asarray(w_h_pw, np.float32)[:, :, 0, 0]       # [O, I]
    wvp = np.asarray(w_v_pw, np.float32)[:, :, 0, 0]
    w1 = np.asarray(w_dm1, np.float32)[:, :, 0, 0]         # [32, 128]
    w2 = np.asarray(w_dm2, np.float32)[:, :, 0, 0]         # [128, 32]
    wf = np.asarray(w_fuse, np.float32)[:, :, 0, 0]        # [128, 256]
    s = float(np.asarray(scale).reshape(-1)[0])

    a_h = s * (wf[:, :P] @ whp)                            # [O, I]
    a_v = s * (wf[:, P:] @ wvp)
    w2s = w2 / s

    wdiag = np.zeros((P, 2 * A, P), np.float32)
    idx = np.arange(P)
    for k in range(A):
        wdiag[idx, k, idx] = wh[:, k]
        wdiag[idx, A + k, idx] = wv[:, k]

    aw = np.stack([a_h.T, a_v.T], axis=1)                  # [c_in, 2, c_out]
    return {
        "wdiag": bf(wdiag),
        "aw": bf(np.ascontiguousarray(aw)),
        "w1t": bf(np.ascontiguousarray(w1.T)),
        "w2t": bf(np.ascontiguousarray(w2s.T)),
    }


def kernel(x, w_h_dw, w_h_pw, w_v_dw, w_v_pw, w_dm1, w_dm2, w_fuse, scale,
           angRes, **_unused):
    x = np.asarray(x, np.float32)
    B, C, H, Wd = x.shape
    assert (B, C, H, Wd) == (2, 128, 320, 320), x.shape
    assert int(np.asarray(angRes)) == A

    s = float(np.asarray(scale).reshape(-1)[0])
    if s == 0.0:
        return x.copy()

    wmap = _prep_weights(w_h_dw, w_h_pw, w_v_dw, w_v_pw, w_dm1, w_dm2, w_fuse, scale)

    # rows of (B, H) flattened: 640 rows; core k takes rows [80k, 80k+80)
    xr = x.reshape(B * H, C, Wd) if False else x  # keep NCHW; slice H per core
    in_maps = []
    for k in range(N_CORES):
        b = k // 4
        r0 = (k % 4) * RPC
        xs = np.ascontiguousarray(x[b, :, r0 : r0 + RPC, :])  # [C, 80, 320]
        m = {"xs": xs}
        m.update(wmap)
        in_maps.append(m)

    nc = _get_nc()
    res = run_bass_kernel_spmd(nc, in_maps, list(range(N_CORES)))

    out = np.empty_like(x)
    for k in range(N_CORES):
        b = k // 4
        r0 = (k % 4) * RPC
        out[b, :, r0 : r0 + RPC, :] = res.results[k]["ys"]
    return out


# revision 7
# speedup vs baseline: 2.1219x; 2.1219x over previous
"""Trainium2 Bass kernel for nn_DepthAwareEPIBranch.

Reference computation (B=2, C=128, H=W=320, angRes=5):
  xe  = angular rearrange: each contiguous 5x5 block of the image is an
        independent "angular patch".
  eh  = pw(lrelu(dwconv_1x5(xe)), w_h_pw)   # taps masked at 5-block bounds
  ev  = pw(lrelu(dwconv_5x1(xe)), w_v_pw)
  epi = pw(concat(eh, ev), w_fuse)
  dw  = sigmoid(pw(lrelu(pw(epi, w_dm1)), w_dm2))
  out = x + scale * epi * dw

Host-side algebraic folds:
  - epi' = scale*epi = A_h @ lrelu(dh) + A_v @ lrelu(dv),
    A_h = scale * w_fuse[:, :C] @ w_h_pw, A_v likewise.
  - dm1 folded past epi:  m1 = B_h @ lrelu(dh) + B_v @ lrelu(dv),
    B_* = w_dm1 @ A_*  (so the epi PSUM tile is only read by the final
    elementwise product, straight from PSUM).
  - w_dm2' = w_dm2 / scale (lrelu is positive-homogeneous), so
    out = x + epi' * sigmoid(...).

Sharding: data-parallel over B*H rows at angular-group granularity:
640 rows = 128 groups of 5; each of 8 cores takes 16 groups (80 rows).

Layout: C=128 = SBUF partitions, pixels on the free dim. Depthwise conv
taps = diagonal-weight bf16 matmuls accumulating in fp32 PSUM. Leaky
relu uses ActivationFunctionType.Prelu (parametric_relu), which lives in
the same ACT table set as Sigmoid -> no table reloads.

Processing unit: a PAIR of groups (10 rows); inner loop r in 0..4
handles row r of both groups so PSUM-reading ops get free dim >= 640.
"""

import numpy as np

import concourse.bacc as bacc
import concourse.mybir as mybir
from concourse import tile
from concourse.bass_utils import run_bass_kernel_spmd

F32 = mybir.dt.float32
BF16 = mybir.dt.bfloat16
AF = mybir.ActivationFunctionType
ALU = mybir.AluOpType

P = 128          # channels = partitions
A = 5            # angRes
W = 320          # image width
NB = W // A      # 64 angular blocks per row
RPC = 80         # rows per core (B*H / 8)
NG = RPC // A    # 16 angular row-groups per core
NPAIR = NG // 2  # 8 pairs
N_CORES = 8

TAPS = [(k, k - 2) for k in range(A)]  # out[j] += w[k] * x[j+k-2]


def _build_nc():
    nc = bacc.Bacc("TRN2", target_bir_lowering=False, debug=False)

    xs = nc.dram_tensor("xs", [P, RPC, W], F32, kind="ExternalInput")
    wdiag = nc.dram_tensor("wdiag", [P, 2 * A, P], BF16, kind="ExternalInput")
    aw = nc.dram_tensor("aw", [P, 2, P], BF16, kind="ExternalInput")   # A_h^T, A_v^T
    bw = nc.dram_tensor("bw", [P, 2, 32], BF16, kind="ExternalInput")  # B_h^T, B_v^T
    w2t4 = nc.dram_tensor("w2t4", [P, P], BF16, kind="ExternalInput")  # stacked W2'^T
    ys = nc.dram_tensor("ys", [P, RPC, W], F32, kind="ExternalOutput")

    with tile.TileContext(nc) as tc:
        with (
            tc.tile_pool(name="consts", bufs=1) as cp,
            tc.tile_pool(name="xin", bufs=2) as xp,
            tc.tile_pool(name="xbf", bufs=2) as xbp,
            tc.tile_pool(name="lrel", bufs=2) as lp,
            tc.tile_pool(name="mid", bufs=3) as mp,
            tc.tile_pool(name="outp", bufs=2) as op,
            tc.tile_pool(name="psc", bufs=2, space="PSUM") as ppc,   # conv dh/dv
            tc.tile_pool(name="pse", bufs=1, space="PSUM") as ppe,   # epi
            tc.tile_pool(name="psm1", bufs=2, space="PSUM") as ppm1,  # dm chain m1/m2
        ):
            wdiag_t = cp.tile([P, 2 * A, P], BF16)
            nc.sync.dma_start(wdiag_t[:], wdiag[:])
            aw_t = cp.tile([P, 2, P], BF16)
            nc.sync.dma_start(aw_t[:], aw[:])
            bw_t = cp.tile([P, 2, 32], BF16)
            nc.sync.dma_start(bw_t[:], bw[:])
            w2t4_t = cp.tile([P, P], BF16)
            nc.sync.dma_start(w2t4_t[:], w2t4[:])

            for pr in range(NPAIR):
                r0 = 2 * A * pr  # first row of the pair (10 rows)
                x_t = xp.tile([P, 2 * A, W], F32, tag="x")
                nc.sync.dma_start(x_t[:], xs[:, r0 : r0 + 2 * A, :])
                xb_t = xbp.tile([P, 2 * A, W], BF16, tag="xb")
                nc.vector.tensor_copy(xb_t[:], x_t[:])
                xbv = xb_t[:].rearrange("p r (b q) -> p r b q", q=A)
                out_t = op.tile([P, 2 * A, W], F32, tag="out")

                for r in range(A):
                    # ---- depthwise convs for row r of both groups -> PSUM
                    dh = ppc.tile([P, 2, 512], F32, tag="conv")
                    for g in range(2):
                        row = g * A + r
                        first = True
                        dhg = dh[:, g, 0:W].rearrange("p (b q) -> p b q", q=A)
                        for k, d in TAPS:
                            if d == 0:
                                o_ap, i_ap = dhg[:, :, :], xbv[:, row, :, :]
                            elif d > 0:
                                o_ap = dhg[:, :, 0 : A - d]
                                i_ap = xbv[:, row, :, d:A]
                            else:
                                o_ap = dhg[:, :, -d:A]
                                i_ap = xbv[:, row, :, 0 : A + d]
                            nc.tensor.matmul(
                                o_ap, wdiag_t[:, k, :], i_ap,
                                start=first, stop=(k == A - 1),
                            )
                            first = False
                    dv = ppc.tile([P, 2, 512], F32, tag="conv")
                    for g in range(2):
                        row = g * A + r
                        vtaps = [(k, d) for k, d in TAPS if 0 <= r + d < A]
                        dvg = dv[:, g, 0:W]
                        for i, (k, d) in enumerate(vtaps):
                            nc.tensor.matmul(
                                dvg, wdiag_t[:, A + k, :],
                                xb_t[:, g * A + r + d, :],
                                start=(i == 0), stop=(i == len(vtaps) - 1),
                            )
                    # ---- leaky relu (Prelu == parametric_relu; sigmoid-set filler)
                    lh = lp.tile([P, 2, W], BF16, tag="lh")
                    nc.scalar.activation(lh[:], dh[:, :, 0:W], AF.Prelu, alpha=0.1)
                    lv = lp.tile([P, 2, W], BF16, tag="lv")
                    nc.scalar.activation(lv[:], dv[:, :, 0:W], AF.Prelu, alpha=0.1)

                    # ---- epi' = A_h @ lh + A_v @ lv (per group; PSUM only)
                    epi = ppe.tile([P, 2, 512], F32, tag="epi")
                    for g in range(2):
                        nc.tensor.matmul(epi[:, g, 0:W], aw_t[:, 0, :], lh[:, g, :],
                                         start=True, stop=False)
                        nc.tensor.matmul(epi[:, g, 0:W], aw_t[:, 1, :], lv[:, g, :],
                                         start=False, stop=True)

                    # ---- depth-weight chain per group: m1 -> lrelu -> m2 -> sigmoid
                    for g in range(2):
                        m1 = ppm1.tile([32, 512], F32, tag="dm")
                        nc.tensor.matmul(m1[:, 0:W], bw_t[:, 0, :], lh[:, g, :],
                                         start=True, stop=False)
                        nc.tensor.matmul(m1[:, 0:W], bw_t[:, 1, :], lv[:, g, :],
                                         start=False, stop=True)
                        lr1 = mp.tile([32, W], BF16, tag="lr1")
                        nc.scalar.activation(lr1[:], m1[:, 0:W], AF.Prelu, alpha=0.1)
                        m2 = ppm1.tile([P, 512], F32, tag="dm")
                        nc.tensor.matmul(m2[:, 0:W], w2t4_t[0:32, :], lr1[:],
                                         start=True, stop=True)
                        sg = mp.tile([P, W], BF16, tag="sg")
                        nc.scalar.activation(sg[:], m2[:, 0:W], AF.Sigmoid)
                        # ---- out = x + epi' * sigmoid
                        prod = mp.tile([P, W], BF16, tag="prod")
                        nc.vector.tensor_tensor(prod[:], epi[:, g, 0:W], sg[:], ALU.mult)
                        row = g * A + r
                        nc.vector.tensor_tensor(
                            out_t[:, row, :], prod[:], x_t[:, row, :], ALU.add
                        )

                nc.sync.dma_start(ys[:, r0 : r0 + 2 * A, :], out_t[:])

    nc.compile()
    return nc


_NC_CACHE = None


def _get_nc():
    global _NC_CACHE
    if _NC_CACHE is None:
        _NC_CACHE = _build_nc()
    return _NC_CACHE


def _prep_weights(w_h_dw, w_h_pw, w_v_dw, w_v_pw, w_dm1, w_dm2, w_fuse, scale):
    """Host-side weight folding; returns the shared per-core weight arrays."""
    import ml_dtypes

    def bf(x):
        return np.ascontiguousarray(np.asarray(x, np.float32)).astype(ml_dtypes.bfloat16)

    wh = np.asarray(w_h_dw, np.float32).reshape(P, A)
    wv = np.asarray(w_v_dw, np.float32).reshape(P, A)
    whp = np.asarray(w_h_pw, np.float32)[:, :, 0, 0]
    wvp = np.asarray(w_v_pw, np.float32)[:, :, 0, 0]
    w1 = np.asarray(w_dm1, np.float32)[:, :, 0, 0]
    w2 = np.asarray(w_dm2, np.float32)[:, :, 0, 0]
    wf = np.asarray(w_fuse, np.float32)[:, :, 0, 0]
    s = float(np.asarray(scale).reshape(-1)[0])

    a_h = s * (wf[:, :P] @ whp)
    a_v = s * (wf[:, P:] @ wvp)
    b_h = w1 @ a_h                       # [32, 128]
    b_v = w1 @ a_v
    w2s = w2 / s                         # [128, 32]

    wdiag = np.zeros((P, 2 * A, P), np.float32)
    idx = np.arange(P)
    for k in range(A):
        wdiag[idx, k, idx] = wh[:, k]
        wdiag[idx, A + k, idx] = wv[:, k]

    w2t4 = np.zeros((P, P), np.float32)
    for j in range(4):
        w2t4[32 * j : 32 * j + 32, :] = w2s.T

    return {
        "wdiag": bf(wdiag),
        "aw": bf(np.stack([a_h.T, a_v.T], axis=1)),
        "bw": bf(np.stack([b_h.T, b_v.T], axis=1)),
        "w2t4": bf(w2t4),
    }


def kernel(x, w_h_dw, w_h_pw, w_v_dw, w_v_pw, w_dm1, w_dm2, w_fuse, scale,
           angRes, **_unused):
    x = np.asarray(x, np.float32)
    B, C, H, Wd = x.shape
    assert (B, C, H, Wd) == (2, 128, 320, 320), x.shape
    assert int(np.asarray(angRes)) == A

    s = float(np.asarray(scale).reshape(-1)[0])
    if s == 0.0:
        return x.copy()

    wmap = _prep_weights(w_h_dw, w_h_pw, w_v_dw, w_v_pw, w_dm1, w_dm2, w_fuse, scale)

    in_maps = []
    for k in range(N_CORES):
        b = k // 4
        r0 = (k % 4) * RPC
        m = {"xs": np.ascontiguousarray(x[b, :, r0 : r0 + RPC, :])}
        m.update(wmap)
        in_maps.append(m)

    nc = _get_nc()
    res = run_bass_kernel_spmd(nc, in_maps, list(range(N_CORES)))

    out = np.empty_like(x)
    for k in range(N_CORES):
        b = k // 4
        r0 = (k % 4) * RPC
        out[b, :, r0 : r0 + RPC, :] = res.results[k]["ys"]
    return out
